# revision 25
# baseline (speedup 1.0000x reference)
"""DKVMN (nn_DKVMN) Trainium2 Bass kernel, data-parallel over batch on 8 cores.

Algorithm (per core, B_local=4, S=2048, M=32, D=64):
  phase A: dma_gather embeddings k=k_emb[skill], v=v_emb[xc]; PE-transpose to
           [d, t] layouts; w = softmax_m(k @ Mk^T) (no max-subtraction: logits
           are tiny); e = sigmoid(v@eW+eb), a = tanh(v@aW+ab) computed directly
           in [d_out, t] layout via PE.
  phase B: the S-step memory recurrence Mv_t = Mv_{t-1}*(1-w⊗e) + w⊗a is an
           independent affine scan per (b,m,d) element; vectorized with the
           DVE tensor_tensor_scan instruction along the time axis.
           Partition layout per tile (b, d_hi): p = m*4+d_lo, free = t.
           Replicated operand tiles built by DMA: E/A from DRAM scratch with
           step-0 outer reads (DRAM source -> the DMA splits across all 16
           SDMA engines by dest partition; an SBUF source with 4 distinct
           rows pins the transfer to 4 engines and port-conflicts the
           reads), W from the SBUF compact (32 source rows, 4x re-read).
           read_t = sum_m w_t*Mv_t via PE matmul with block-ones lhsT,
           accumulated over the 16 d_hi tiles into one PSUM [64 d, t] tile.
  phase C: f = tanh([read, k] @ f_W + f_b) as two accumulating PE matmuls
           (f_W split), p = f @ p_W + p_b. Output p[:, 1:].
Mask support: mask is folded multiplicatively into e and a compacts
(m in {0,1}: m=0 -> alpha=1, beta=0 -> state unchanged, matching jnp.where).
The fold is only built when the host sees a non-trivial mask.
"""

import sys

sys.path.insert(0, "/opt/trn_rl_repo")

import numpy as np
import ml_dtypes

import concourse.bass as bass
import concourse.bacc as bacc
import concourse.tile as tile
from concourse import mybir

F32 = mybir.dt.float32
BF16 = mybir.dt.bfloat16
I16 = mybir.dt.int16
AF = mybir.ActivationFunctionType
OP = mybir.AluOpType
BF = ml_dtypes.bfloat16

B, S, NSKILL, D, M = 32, 2048, 4096, 64, 32
NCORES = 8
BL = B // NCORES  # 4 local batches per core
NTOK = BL * S  # 8192 tokens per core
NG = NTOK // 128  # 64 gather groups of 128 tokens
GPB = S // 128  # 16 groups per batch
NWIN = S // 512  # 4 psum windows per batch
NDH = D // 4  # 16 d_hi tiles per batch


def _build(masked: bool):
    nc = bacc.Bacc("TRN2", target_bir_lowering=False, debug=False)

    def din(name, shape, dt):
        return nc.dram_tensor(name, shape, dt, kind="ExternalInput").ap()

    xc_d = din("xc_idx", (128, NG), mybir.dt.int32)
    kvemb_d = din("kv_emb", (2 * NSKILL, 2 * D), F32)
    mkT_d = din("MkT", (D, M), BF16)
    eW_d = din("eW", (D, D), BF16)
    aW_d = din("aW", (D, D), BF16)
    fW1_d = din("fW1", (D, D), BF16)
    fW2_d = din("fW2", (D, D), BF16)
    pW_d = din("pW", (D, 1), BF16)
    eb_d = din("eb", (D, 1), F32)
    ab_d = din("ab", (D, 1), F32)
    fb_d = din("fb", (D, 1), F32)
    pb_d = din("pb", (1, 1), F32)
    mv0_d = din("mv0i", (128, NDH), F32)
    ones_d = din("ones_red", (128, NDH * D), BF16)
    id_d = din("ident", (128, 128), F32)
    mask_d = din("maskT", (BL, S), BF16) if masked else None
    out_d = nc.dram_tensor("out", (BL, S - 1), F32, kind="ExternalOutput").ap()

    e_dram = nc.dram_tensor("e_scratch", (D, NTOK), BF16).ap()
    a_dram = nc.dram_tensor("a_scratch", (D, NTOK), BF16).ap()

    with tile.TileContext(nc) as tc:
        with (
            tc.tile_pool(name="const", bufs=1) as cpool,
            tc.tile_pool(name="persist", bufs=1) as ppool,
        ):
            # ---- constants to SBUF ----
            xcid = cpool.tile([128, NG], mybir.dt.int32)
            mkT = cpool.tile([D, M], BF16)
            eW = cpool.tile([D, D], BF16)
            aW = cpool.tile([D, D], BF16)
            fW1 = cpool.tile([D, D], BF16)
            fW2 = cpool.tile([D, D], BF16)
            pW = cpool.tile([D, 1], BF16)
            eb = cpool.tile([D, 1], F32)
            ab = cpool.tile([D, 1], F32)
            fb = cpool.tile([D, 1], F32)
            pb = cpool.tile([1, 1], F32)
            mv0 = cpool.tile([128, NDH], F32)
            ones = cpool.tile([128, NDH * D], BF16)
            idt = cpool.tile([128, 128], F32)
            for t_, d_ in [
                (xcid, xc_d), (mkT, mkT_d), (eW, eW_d), (aW, aW_d),
                (fW1, fW1_d), (fW2, fW2_d), (pW, pW_d), (eb, eb_d), (ab, ab_d),
                (fb, fb_d), (pb, pb_d), (mv0, mv0_d), (ones, ones_d), (idt, id_d),
            ]:
                nc.sync.dma_start(t_[:], d_[:])

            # ---- persistent compacts ----
            kT = ppool.tile([D, NTOK], BF16)  # [d, (b,t)]
            wC = ppool.tile([M, NTOK], BF16)  # [m, (b,t)]
            eC = ppool.tile([D, NTOK], BF16)
            aC = ppool.tile([D, NTOK], BF16)
            p_sb = [
                ppool.tile([1, S], F32, tag=f"psb{b}", name=f"psb{b}")
                for b in range(BL)
            ]

            # ================= phase A =================
            with (
                tc.tile_pool(name="gat", bufs=1) as gpool,
                tc.tile_pool(name="aw", bufs=3) as awpool,
                tc.tile_pool(name="apsT", bufs=2, space="PSUM") as apsT,
                tc.tile_pool(name="apsW", bufs=1, space="PSUM") as apsW,
                tc.tile_pool(name="apsE", bufs=1, space="PSUM") as apsE,
            ):
                kvg = gpool.tile([128, NG, 2 * D], F32)
                for g in range(NG):
                    nc.gpsimd.indirect_dma_start(
                        out=kvg[:, g, :],
                        out_offset=None,
                        in_=kvemb_d[:],
                        in_offset=bass.IndirectOffsetOnAxis(
                            ap=xcid[:, g : g + 1], axis=0
                        ),
                    )
                vT = gpool.tile([D, NTOK], BF16)

                # transpose kv rows to [d, tok]; 4 groups per PSUM tile
                for q in range(NG // 4):
                    tp = apsT.tile([128, 512], F32)
                    for j in range(4):
                        g = q * 4 + j
                        nc.tensor.transpose(
                            tp[:, j * 128 : (j + 1) * 128], kvg[:, g, :], idt[:]
                        )
                    nc.scalar.activation(
                        kT[:, q * 512 : (q + 1) * 512], tp[0:D, :], AF.Copy
                    )
                    nc.scalar.activation(
                        vT[:, q * 512 : (q + 1) * 512], tp[D : 2 * D, :], AF.Copy
                    )

                # w path per batch: logits -> exp -> segmented sum -> normalize
                for b in range(BL):
                    lg = apsW.tile([128, GPB * M], F32)  # [tok128, (g, m)]
                    for g in range(GPB):
                        tok0 = (b * GPB + g) * 128
                        nc.tensor.matmul(
                            lg[:, g * M : (g + 1) * M],
                            kT[:, tok0 : tok0 + 128],
                            mkT[:],
                            start=True, stop=True,
                        )
                    wexp = awpool.tile([128, GPB * M], F32, tag="wexp")
                    nc.scalar.activation(wexp[:], lg[:], AF.Exp)
                    ssum = awpool.tile([128, GPB], F32, tag="ssum")
                    nc.vector.tensor_reduce(
                        ssum[:],
                        wexp[:].rearrange("p (g m) -> p g m", g=GPB),
                        mybir.AxisListType.X,
                        OP.add,
                    )
                    rec = awpool.tile([128, GPB], F32, tag="rec")
                    nc.vector.reciprocal(rec[:], ssum[:])
                    wn = awpool.tile([128, GPB * M], F32, tag="wn")
                    nc.vector.tensor_tensor(
                        wn[:].rearrange("p (g m) -> p g m", g=GPB),
                        wexp[:].rearrange("p (g m) -> p g m", g=GPB),
                        rec[:].unsqueeze(2).broadcast_to([128, GPB, M]),
                        OP.mult,
                    )
                    # transpose w -> [m, t], cast bf16 into the persistent compact
                    for q in range(GPB // 4):
                        tpw = apsT.tile([M, 512], F32, tag="tpw")
                        for j in range(4):
                            g = q * 4 + j
                            nc.tensor.transpose(
                                tpw[:, j * 128 : (j + 1) * 128],
                                wn[:, g * M : (g + 1) * M],
                                idt[:],
                            )
                        nc.scalar.activation(
                            wC[:, b * S + q * 512 : b * S + (q + 1) * 512],
                            tpw[:],
                            AF.Copy,
                        )

                # e, a in [d_out, t] layout directly
                for b in range(BL):
                    for win in range(NWIN):
                        c0 = b * S + win * 512
                        eps = apsE.tile([D, 512], F32, tag="eps")
                        nc.tensor.matmul(
                            eps[:], eW[:], vT[:, c0 : c0 + 512], start=True, stop=True
                        )
                        nc.scalar.activation(
                            eC[:, c0 : c0 + 512], eps[:], AF.Sigmoid, bias=eb[:]
                        )
                        aps_ = apsE.tile([D, 512], F32, tag="aps")
                        nc.tensor.matmul(
                            aps_[:], aW[:], vT[:, c0 : c0 + 512], start=True, stop=True
                        )
                        nc.scalar.activation(
                            aC[:, c0 : c0 + 512], aps_[:], AF.Tanh, bias=ab[:]
                        )

                if masked:
                    for b in range(BL):
                        mrep = awpool.tile([D, S], BF16, tag="mrep")
                        nc.sync.dma_start(
                            mrep[:],
                            mask_d[b : b + 1, :].broadcast_to([D, S]),
                        )
                        c0 = b * S
                        nc.vector.tensor_tensor(
                            eC[:, c0 : c0 + S], eC[:, c0 : c0 + S], mrep[:], OP.mult
                        )
                        nc.vector.tensor_tensor(
                            aC[:, c0 : c0 + S], aC[:, c0 : c0 + S], mrep[:], OP.mult
                        )

                # spill e/a compacts to DRAM per batch (phase-B replication
                # DMAs read from DRAM so each splits across all 16 SDMA
                # engines; per-b spills let b=0's phase B start while phase A
                # still works on b>0)
                for b in range(BL):
                    nc.sync.dma_start(
                        e_dram[:, b * S : (b + 1) * S], eC[:, b * S : (b + 1) * S]
                    )
                    nc.sync.dma_start(
                        a_dram[:, b * S : (b + 1) * S], aC[:, b * S : (b + 1) * S]
                    )

            # ================= phases B + C =================
            with (
                tc.tile_pool(name="scan", bufs=2) as bpool,
                tc.tile_pool(name="wrp", bufs=2) as wrpool,
                tc.tile_pool(name="cc", bufs=2) as ccpool,
                tc.tile_pool(name="bpsR", bufs=1, space="PSUM") as bpsR,
                tc.tile_pool(name="bpsF", bufs=2, space="PSUM") as bpsF,
                tc.tile_pool(name="bpsP", bufs=2, space="PSUM") as bpsP,
            ):
                for b in range(BL):
                    # W_rep[(m,dlo), t] = w[m, t]: SBUF source, 32 rows x4
                    wr = wrpool.tile([128, S], BF16, tag="wr")
                    nc.sync.dma_start(
                        wr[:],
                        wC[:, b * S : (b + 1) * S]
                        .unsqueeze(1)
                        .broadcast_to([M, 4, S]),
                    )
                    rps = bpsR.tile([D, S], F32)  # read accumulator [d, t]
                    for dh in range(NDH):
                        c0 = b * S
                        er = bpool.tile([128, S], BF16, tag="er")
                        nc.scalar.dma_start(
                            er[:],
                            e_dram[dh * 4 : dh * 4 + 4, c0 : c0 + S]
                            .unsqueeze(0)
                            .broadcast_to([M, 4, S]),
                        )
                        ar = bpool.tile([128, S], BF16, tag="ar")
                        nc.sync.dma_start(
                            ar[:],
                            a_dram[dh * 4 : dh * 4 + 4, c0 : c0 + S]
                            .unsqueeze(0)
                            .broadcast_to([M, 4, S]),
                        )
                        we = bpool.tile([128, S], BF16, tag="we")
                        nc.vector.tensor_tensor(we[:], wr[:], er[:], OP.mult)
                        al = bpool.tile([128, S], BF16, tag="al")
                        nc.scalar.activation(al[:], we[:], AF.Copy, bias=1.0, scale=-1.0)
                        be = bpool.tile([128, S], BF16, tag="be")
                        # give half the beta products to the otherwise-idle
                        # GPSIMD (walrus rejects the scan itself on Pool)
                        be_eng = nc.gpsimd if dh % 2 == 1 else nc.vector
                        be_eng.tensor_tensor(be[:], wr[:], ar[:], OP.mult)
                        mv = bpool.tile([128, S], BF16, tag="mv")
                        nc.vector.tensor_tensor_scan(
                            mv[:], al[:], be[:], mv0[:, dh : dh + 1], OP.mult, OP.add
                        )
                        wm = bpool.tile([128, S], BF16, tag="wm")
                        nc.vector.tensor_tensor(wm[:], wr[:], mv[:], OP.mult)
                        for win in range(NWIN):
                            nc.tensor.matmul(
                                rps[:, win * 512 : (win + 1) * 512],
                                ones[:, dh * D : (dh + 1) * D],
                                wm[:, win * 512 : (win + 1) * 512],
                                start=(dh == 0),
                                stop=(dh == NDH - 1),
                            )
                    # ---- phase C for this batch ----
                    for win in range(NWIN):
                        rd = ccpool.tile([D, 512], BF16, tag="rd")
                        nc.scalar.activation(
                            rd[:], rps[:, win * 512 : (win + 1) * 512], AF.Copy
                        )
                        fps = bpsF.tile([D, 512], F32, tag="fps")
                        nc.tensor.matmul(fps[:], fW1[:], rd[:], start=True, stop=False)
                        nc.tensor.matmul(
                            fps[:],
                            fW2[:],
                            kT[:, b * S + win * 512 : b * S + (win + 1) * 512],
                            start=False,
                            stop=True,
                        )
                        fsb = ccpool.tile([D, 512], BF16, tag="fsb")
                        nc.scalar.activation(fsb[:], fps[:], AF.Tanh, bias=fb[:])
                        pps = bpsP.tile([1, 512], F32, tag="pps")
                        nc.tensor.matmul(pps[:], pW[:], fsb[:], start=True, stop=True)
                        nc.scalar.activation(
                            p_sb[b][:, win * 512 : (win + 1) * 512],
                            pps[:],
                            AF.Identity,
                            bias=pb[:],
                        )
                    nc.sync.dma_start(out_d[b : b + 1, :], p_sb[b][:, 1:S])

    nc.compile()
    return nc


_CACHE: dict = {}


def _get_nc(masked: bool):
    if masked not in _CACHE:
        _CACHE[masked] = _build(masked)
    return _CACHE[masked]


def _wrap_idx(idx):
    """[BL, S] int -> per-group indirect-DMA offsets [128, NG] int32.

    out[p, g] = flat token index g*128+p (token t lands at partition t%128)."""
    flat = np.ascontiguousarray(idx).reshape(-1).astype(np.int32)
    return np.ascontiguousarray(flat.reshape(NG, 128).T)


def _prep_in_maps(inputs):
    skill = np.asarray(inputs["skill"]).astype(np.int64)
    response = np.asarray(inputs["response"]).astype(np.int64)
    mask = np.asarray(inputs["mask"]).astype(np.float32)
    xc = skill + NSKILL * response

    mv0 = np.asarray(inputs["Mv0"], np.float32)  # [M, D]
    mv0i = np.zeros((128, NDH), np.float32)
    ones = np.zeros((128, NDH * D), np.float32)
    for m in range(M):
        for dlo in range(4):
            p = m * 4 + dlo
            mv0i[p, :] = mv0[m, dlo::4]
            for dh in range(NDH):
                ones[p, dh * D + dh * 4 + dlo] = 1.0

    f_W = np.asarray(inputs["f_W"], np.float32)
    kce = np.asarray(inputs["k_c_emb"], np.float32)
    vce = np.asarray(inputs["v_c_emb"], np.float32)
    kv = np.empty((2 * NSKILL, 2 * D), np.float32)
    kv[:NSKILL, :D] = kce
    kv[NSKILL:, :D] = kce
    kv[:, D:] = vce
    shared = {
        "kv_emb": kv,
        "MkT": np.asarray(inputs["Mk"], np.float32).T.astype(BF),
        "eW": np.asarray(inputs["e_W"], np.float32).astype(BF),
        "aW": np.asarray(inputs["a_W"], np.float32).astype(BF),
        "fW1": f_W[:D].astype(BF),
        "fW2": f_W[D:].astype(BF),
        "pW": np.asarray(inputs["p_W"], np.float32).astype(BF),
        "eb": np.asarray(inputs["e_b"], np.float32).reshape(D, 1),
        "ab": np.asarray(inputs["a_b"], np.float32).reshape(D, 1),
        "fb": np.asarray(inputs["f_b"], np.float32).reshape(D, 1),
        "pb": np.asarray(inputs["p_b"], np.float32).reshape(1, 1),
        "mv0i": mv0i,
        "ones_red": ones.astype(BF),
        "ident": np.eye(128, dtype=np.float32),
    }
    masked = not np.all(mask == 1.0)
    in_maps = []
    for c in range(NCORES):
        sl = slice(c * BL, (c + 1) * BL)
        im = dict(shared)
        im["xc_idx"] = _wrap_idx(xc[sl])
        if masked:
            im["maskT"] = mask[sl].astype(BF)
        in_maps.append(im)
    return in_maps, masked


def kernel(**inputs) -> np.ndarray:
    from concourse.bass_utils import run_bass_kernel_spmd

    in_maps, masked = _prep_in_maps(inputs)
    nc = _get_nc(masked)
    res = run_bass_kernel_spmd(nc, in_maps, list(range(NCORES)))
    out = np.concatenate([np.asarray(res.results[c]["out"]) for c in range(NCORES)], 0)
    return out.astype(np.float32)


# revision 26
# speedup vs baseline: 1.1398x; 1.1398x over previous
"""DKVMN (nn_DKVMN) Trainium2 Bass kernel, data-parallel over batch on 8 cores.

Algorithm (per core, B_local=4, S=2048, M=32, D=64):
  phase A: dma_gather embeddings k=k_emb[skill], v=v_emb[xc]; PE-transpose to
           [d, t] layouts; w = softmax_m(k @ Mk^T) (no max-subtraction: logits
           are tiny); e = sigmoid(v@eW+eb), a = tanh(v@aW+ab) computed directly
           in [d_out, t] layout via PE.
  phase B: the S-step memory recurrence Mv_t = Mv_{t-1}*(1-w⊗e) + w⊗a is an
           independent affine scan per (b,m,d) element; vectorized with the
           DVE tensor_tensor_scan instruction along the time axis.
           Partition layout per tile (b, d_hi): p = m*4+d_lo, free = t.
           Replicated operand tiles built by DMA: E/A from DRAM scratch with
           step-0 outer reads (DRAM source -> the DMA splits across all 16
           SDMA engines by dest partition; an SBUF source with 4 distinct
           rows pins the transfer to 4 engines and port-conflicts the
           reads), W from the SBUF compact (32 source rows, 4x re-read).
           read_t = sum_m w_t*Mv_t via PE matmul with block-ones lhsT,
           accumulated over the 16 d_hi tiles into one PSUM [64 d, t] tile.
  phase C: f = tanh([read, k] @ f_W + f_b) as two accumulating PE matmuls
           (f_W split), p = f @ p_W + p_b. Output p[:, 1:].
Mask support: mask is folded multiplicatively into e and a compacts
(m in {0,1}: m=0 -> alpha=1, beta=0 -> state unchanged, matching jnp.where).
The fold is only built when the host sees a non-trivial mask.
"""

import sys

sys.path.insert(0, "/opt/trn_rl_repo")

import numpy as np
import ml_dtypes

import concourse.bass as bass
import concourse.bacc as bacc
import concourse.tile as tile
from concourse import mybir

F32 = mybir.dt.float32
BF16 = mybir.dt.bfloat16
I16 = mybir.dt.int16
AF = mybir.ActivationFunctionType
OP = mybir.AluOpType
BF = ml_dtypes.bfloat16

B, S, NSKILL, D, M = 32, 2048, 4096, 64, 32
NCORES = 8
BL = B // NCORES  # 4 local batches per core
NTOK = BL * S  # 8192 tokens per core
NG = NTOK // 128  # 64 gather groups of 128 tokens
GPB = S // 128  # 16 groups per batch
NWIN = S // 512  # 4 psum windows per batch
NDH = D // 4  # 16 d_hi tiles per batch


def _build(masked: bool):
    nc = bacc.Bacc("TRN2", target_bir_lowering=False, debug=False)

    def din(name, shape, dt):
        return nc.dram_tensor(name, shape, dt, kind="ExternalInput").ap()

    xc_d = din("xc_idx", (128, NG), mybir.dt.int32)
    kvemb_d = din("kv_emb", (2 * NSKILL, 2 * D), F32)
    mkT_d = din("MkT", (D, M), BF16)
    eW_d = din("eW", (D, D), BF16)
    aW_d = din("aW", (D, D), BF16)
    fW1_d = din("fW1", (D, D), BF16)
    fW2_d = din("fW2", (D, D), BF16)
    pW_d = din("pW", (D, 1), BF16)
    eb_d = din("eb", (D, 1), F32)
    ab_d = din("ab", (D, 1), F32)
    fb_d = din("fb", (D, 1), F32)
    pb_d = din("pb", (1, 1), F32)
    mv0_d = din("mv0i", (128, NDH), F32)
    ones_d = din("ones_red", (128, NDH * D), BF16)
    id_d = din("ident", (128, 128), F32)
    mask_d = din("maskT", (BL, S), BF16) if masked else None
    out_d = nc.dram_tensor("out", (BL, S - 1), F32, kind="ExternalOutput").ap()

    e_dram = nc.dram_tensor("e_scratch", (D, NTOK), BF16).ap()
    a_dram = nc.dram_tensor("a_scratch", (D, NTOK), BF16).ap()

    with tile.TileContext(nc) as tc:
        with (
            tc.tile_pool(name="const", bufs=1) as cpool,
            tc.tile_pool(name="persist", bufs=1) as ppool,
        ):
            # ---- constants to SBUF ----
            xcid = cpool.tile([128, NG], mybir.dt.int32)
            mkT = cpool.tile([D, M], BF16)
            eW = cpool.tile([D, D], BF16)
            aW = cpool.tile([D, D], BF16)
            fW1 = cpool.tile([D, D], BF16)
            fW2 = cpool.tile([D, D], BF16)
            pW = cpool.tile([D, 1], BF16)
            eb = cpool.tile([D, 1], F32)
            ab = cpool.tile([D, 1], F32)
            fb = cpool.tile([D, 1], F32)
            pb = cpool.tile([1, 1], F32)
            mv0 = cpool.tile([128, NDH], F32)
            ones = cpool.tile([128, NDH * D], BF16)
            idt = cpool.tile([128, 128], F32)
            for t_, d_ in [
                (xcid, xc_d), (mkT, mkT_d), (eW, eW_d), (aW, aW_d),
                (fW1, fW1_d), (fW2, fW2_d), (pW, pW_d), (eb, eb_d), (ab, ab_d),
                (fb, fb_d), (pb, pb_d), (mv0, mv0_d), (ones, ones_d), (idt, id_d),
            ]:
                nc.sync.dma_start(t_[:], d_[:])

            # ---- persistent compacts ----
            kT = ppool.tile([D, NTOK], BF16)  # [d, (b,t)]
            wC = ppool.tile([M, NTOK], BF16)  # [m, (b,t)]
            eC = ppool.tile([D, NTOK], BF16)
            aC = ppool.tile([D, NTOK], BF16)
            p_sb = [
                ppool.tile([1, S], F32, tag=f"psb{b}", name=f"psb{b}")
                for b in range(BL)
            ]

            # ================= phase A =================
            with (
                tc.tile_pool(name="gat", bufs=1) as gpool,
                tc.tile_pool(name="aw", bufs=3) as awpool,
                tc.tile_pool(name="apsT", bufs=2, space="PSUM") as apsT,
                tc.tile_pool(name="apsW", bufs=1, space="PSUM") as apsW,
                tc.tile_pool(name="apsE", bufs=1, space="PSUM") as apsE,
            ):
                kvg = gpool.tile([128, NG, 2 * D], F32)
                for g in range(NG):
                    nc.gpsimd.indirect_dma_start(
                        out=kvg[:, g, :],
                        out_offset=None,
                        in_=kvemb_d[:],
                        in_offset=bass.IndirectOffsetOnAxis(
                            ap=xcid[:, g : g + 1], axis=0
                        ),
                    )
                vT = gpool.tile([D, NTOK], BF16)

                # transpose kv rows to [d, tok]; 4 groups per PSUM tile
                for q in range(NG // 4):
                    tp = apsT.tile([128, 512], F32)
                    for j in range(4):
                        g = q * 4 + j
                        nc.tensor.transpose(
                            tp[:, j * 128 : (j + 1) * 128], kvg[:, g, :], idt[:]
                        )
                    nc.scalar.activation(
                        kT[:, q * 512 : (q + 1) * 512], tp[0:D, :], AF.Copy
                    )
                    nc.scalar.activation(
                        vT[:, q * 512 : (q + 1) * 512], tp[D : 2 * D, :], AF.Copy
                    )

                # w path per batch: logits -> exp -> segmented sum -> normalize
                for b in range(BL):
                    lg = apsW.tile([128, GPB * M], F32)  # [tok128, (g, m)]
                    for g in range(GPB):
                        tok0 = (b * GPB + g) * 128
                        nc.tensor.matmul(
                            lg[:, g * M : (g + 1) * M],
                            kT[:, tok0 : tok0 + 128],
                            mkT[:],
                            start=True, stop=True,
                        )
                    wexp = awpool.tile([128, GPB * M], F32, tag="wexp")
                    nc.scalar.activation(wexp[:], lg[:], AF.Exp)
                    ssum = awpool.tile([128, GPB], F32, tag="ssum")
                    nc.vector.tensor_reduce(
                        ssum[:],
                        wexp[:].rearrange("p (g m) -> p g m", g=GPB),
                        mybir.AxisListType.X,
                        OP.add,
                    )
                    rec = awpool.tile([128, GPB], F32, tag="rec")
                    nc.vector.reciprocal(rec[:], ssum[:])
                    wn = awpool.tile([128, GPB * M], F32, tag="wn")
                    nc.vector.tensor_tensor(
                        wn[:].rearrange("p (g m) -> p g m", g=GPB),
                        wexp[:].rearrange("p (g m) -> p g m", g=GPB),
                        rec[:].unsqueeze(2).broadcast_to([128, GPB, M]),
                        OP.mult,
                    )
                    # transpose w -> [m, t], cast bf16 into the persistent compact
                    for q in range(GPB // 4):
                        tpw = apsT.tile([M, 512], F32, tag="tpw")
                        for j in range(4):
                            g = q * 4 + j
                            nc.tensor.transpose(
                                tpw[:, j * 128 : (j + 1) * 128],
                                wn[:, g * M : (g + 1) * M],
                                idt[:],
                            )
                        nc.scalar.activation(
                            wC[:, b * S + q * 512 : b * S + (q + 1) * 512],
                            tpw[:],
                            AF.Copy,
                        )

                # e, a in [d_out, t] layout directly
                for b in range(BL):
                    for win in range(NWIN):
                        c0 = b * S + win * 512
                        eps = apsE.tile([D, 512], F32, tag="eps")
                        nc.tensor.matmul(
                            eps[:], eW[:], vT[:, c0 : c0 + 512], start=True, stop=True
                        )
                        nc.scalar.activation(
                            eC[:, c0 : c0 + 512], eps[:], AF.Sigmoid, bias=eb[:]
                        )
                        aps_ = apsE.tile([D, 512], F32, tag="aps")
                        nc.tensor.matmul(
                            aps_[:], aW[:], vT[:, c0 : c0 + 512], start=True, stop=True
                        )
                        nc.scalar.activation(
                            aC[:, c0 : c0 + 512], aps_[:], AF.Tanh, bias=ab[:]
                        )

                if masked:
                    for b in range(BL):
                        mrep = awpool.tile([D, S], BF16, tag="mrep")
                        nc.sync.dma_start(
                            mrep[:],
                            mask_d[b : b + 1, :].broadcast_to([D, S]),
                        )
                        c0 = b * S
                        nc.vector.tensor_tensor(
                            eC[:, c0 : c0 + S], eC[:, c0 : c0 + S], mrep[:], OP.mult
                        )
                        nc.vector.tensor_tensor(
                            aC[:, c0 : c0 + S], aC[:, c0 : c0 + S], mrep[:], OP.mult
                        )

                # spill e/a compacts to DRAM per batch (phase-B replication
                # DMAs read from DRAM so each splits across all 16 SDMA
                # engines; per-b spills let b=0's phase B start while phase A
                # still works on b>0)
                for b in range(BL):
                    nc.sync.dma_start(
                        e_dram[:, b * S : (b + 1) * S], eC[:, b * S : (b + 1) * S]
                    )
                    nc.sync.dma_start(
                        a_dram[:, b * S : (b + 1) * S], aC[:, b * S : (b + 1) * S]
                    )

            # ================= phases B + C =================
            with (
                tc.tile_pool(name="scan", bufs=2) as bpool,
                tc.tile_pool(name="wrp", bufs=2) as wrpool,
                tc.tile_pool(name="cc", bufs=2) as ccpool,
                tc.tile_pool(name="bpsR", bufs=1, space="PSUM") as bpsR,
                tc.tile_pool(name="bpsF", bufs=2, space="PSUM") as bpsF,
                tc.tile_pool(name="bpsP", bufs=2, space="PSUM") as bpsP,
            ):
                for b in range(BL):
                    # W_rep[(m,dlo), t] = w[m, t]: SBUF source, 32 rows x4
                    wr = wrpool.tile([128, S], BF16, tag="wr")
                    nc.sync.dma_start(
                        wr[:],
                        wC[:, b * S : (b + 1) * S]
                        .unsqueeze(1)
                        .broadcast_to([M, 4, S]),
                    )
                    rps = bpsR.tile([D, S], F32)  # read accumulator [d, t]
                    for dh in range(NDH):
                        c0 = b * S
                        er = bpool.tile([128, S], BF16, tag="er")
                        nc.scalar.dma_start(
                            er[:],
                            e_dram[dh * 4 : dh * 4 + 4, c0 : c0 + S]
                            .unsqueeze(0)
                            .broadcast_to([M, 4, S]),
                        )
                        ar = bpool.tile([128, S], BF16, tag="ar")
                        nc.sync.dma_start(
                            ar[:],
                            a_dram[dh * 4 : dh * 4 + 4, c0 : c0 + S]
                            .unsqueeze(0)
                            .broadcast_to([M, 4, S]),
                        )
                        we = bpool.tile([128, S], BF16, tag="we")
                        nc.vector.tensor_tensor(we[:], wr[:], er[:], OP.mult)
                        al = bpool.tile([128, S], BF16, tag="al")
                        nc.scalar.activation(al[:], we[:], AF.Copy, bias=1.0, scale=-1.0)
                        be = bpool.tile([128, S], BF16, tag="be")
                        nc.vector.tensor_tensor(be[:], wr[:], ar[:], OP.mult)
                        mv = bpool.tile([128, S], BF16, tag="mv")
                        nc.vector.tensor_tensor_scan(
                            mv[:], al[:], be[:], mv0[:, dh : dh + 1], OP.mult, OP.add
                        )
                        wm = bpool.tile([128, S], BF16, tag="wm")
                        nc.vector.tensor_tensor(wm[:], wr[:], mv[:], OP.mult)
                        for win in range(NWIN):
                            nc.tensor.matmul(
                                rps[:, win * 512 : (win + 1) * 512],
                                ones[:, dh * D : (dh + 1) * D],
                                wm[:, win * 512 : (win + 1) * 512],
                                start=(dh == 0),
                                stop=(dh == NDH - 1),
                            )
                    # ---- phase C for this batch ----
                    for win in range(NWIN):
                        rd = ccpool.tile([D, 512], BF16, tag="rd")
                        nc.scalar.activation(
                            rd[:], rps[:, win * 512 : (win + 1) * 512], AF.Copy
                        )
                        fps = bpsF.tile([D, 512], F32, tag="fps")
                        nc.tensor.matmul(fps[:], fW1[:], rd[:], start=True, stop=False)
                        nc.tensor.matmul(
                            fps[:],
                            fW2[:],
                            kT[:, b * S + win * 512 : b * S + (win + 1) * 512],
                            start=False,
                            stop=True,
                        )
                        fsb = ccpool.tile([D, 512], BF16, tag="fsb")
                        nc.scalar.activation(fsb[:], fps[:], AF.Tanh, bias=fb[:])
                        pps = bpsP.tile([1, 512], F32, tag="pps")
                        nc.tensor.matmul(pps[:], pW[:], fsb[:], start=True, stop=True)
                        nc.scalar.activation(
                            p_sb[b][:, win * 512 : (win + 1) * 512],
                            pps[:],
                            AF.Identity,
                            bias=pb[:],
                        )
                    nc.sync.dma_start(out_d[b : b + 1, :], p_sb[b][:, 1:S])

    nc.compile()
    return nc


_CACHE: dict = {}


def _get_nc(masked: bool):
    if masked not in _CACHE:
        _CACHE[masked] = _build(masked)
    return _CACHE[masked]


def _wrap_idx(idx):
    """[BL, S] int -> per-group indirect-DMA offsets [128, NG] int32.

    out[p, g] = flat token index g*128+p (token t lands at partition t%128)."""
    flat = np.ascontiguousarray(idx).reshape(-1).astype(np.int32)
    return np.ascontiguousarray(flat.reshape(NG, 128).T)


def _prep_in_maps(inputs):
    skill = np.asarray(inputs["skill"]).astype(np.int64)
    response = np.asarray(inputs["response"]).astype(np.int64)
    mask = np.asarray(inputs["mask"]).astype(np.float32)
    xc = skill + NSKILL * response

    mv0 = np.asarray(inputs["Mv0"], np.float32)  # [M, D]
    mv0i = np.zeros((128, NDH), np.float32)
    ones = np.zeros((128, NDH * D), np.float32)
    for m in range(M):
        for dlo in range(4):
            p = m * 4 + dlo
            mv0i[p, :] = mv0[m, dlo::4]
            for dh in range(NDH):
                ones[p, dh * D + dh * 4 + dlo] = 1.0

    f_W = np.asarray(inputs["f_W"], np.float32)
    kce = np.asarray(inputs["k_c_emb"], np.float32)
    vce = np.asarray(inputs["v_c_emb"], np.float32)
    kv = np.empty((2 * NSKILL, 2 * D), np.float32)
    kv[:NSKILL, :D] = kce
    kv[NSKILL:, :D] = kce
    kv[:, D:] = vce
    shared = {
        "kv_emb": kv,
        "MkT": np.asarray(inputs["Mk"], np.float32).T.astype(BF),
        "eW": np.asarray(inputs["e_W"], np.float32).astype(BF),
        "aW": np.asarray(inputs["a_W"], np.float32).astype(BF),
        "fW1": f_W[:D].astype(BF),
        "fW2": f_W[D:].astype(BF),
        "pW": np.asarray(inputs["p_W"], np.float32).astype(BF),
        "eb": np.asarray(inputs["e_b"], np.float32).reshape(D, 1),
        "ab": np.asarray(inputs["a_b"], np.float32).reshape(D, 1),
        "fb": np.asarray(inputs["f_b"], np.float32).reshape(D, 1),
        "pb": np.asarray(inputs["p_b"], np.float32).reshape(1, 1),
        "mv0i": mv0i,
        "ones_red": ones.astype(BF),
        "ident": np.eye(128, dtype=np.float32),
    }
    masked = not np.all(mask == 1.0)
    in_maps = []
    for c in range(NCORES):
        sl = slice(c * BL, (c + 1) * BL)
        im = dict(shared)
        im["xc_idx"] = _wrap_idx(xc[sl])
        if masked:
            im["maskT"] = mask[sl].astype(BF)
        in_maps.append(im)
    return in_maps, masked


def kernel(**inputs) -> np.ndarray:
    from concourse.bass_utils import run_bass_kernel_spmd

    in_maps, masked = _prep_in_maps(inputs)
    nc = _get_nc(masked)
    res = run_bass_kernel_spmd(nc, in_maps, list(range(NCORES)))
    out = np.concatenate([np.asarray(res.results[c]["out"]) for c in range(NCORES)], 0)
    return out.astype(np.float32)
